# revision 20
# baseline (speedup 1.0000x reference)
"""Trainium2 Bass kernel for nn_Minimax_Conv2D.

Semantics (reference): for each output channel o and pixel (b,h,w):
    v_j = x_padEdge[b, c_j, h+kh_j, w+kw_j]   (c_j,kh_j,kw_j) = decode(conn[o*9+j])
    out  = min_i max_{j in triple i} (v_j - w1[o,j]) - w2[o,i]

Strategy:
  - 8-way data parallel over batch (2 batches/core), identical SPMD program.
  - Per core SBUF layout: partitions p = b_local*64 + h ; free = (dh, c, w_pad)
    holding 3 h-shifted edge-padded copies of the input, so every gather
    offset (c, kh, kw) is a static free-dim slice baked at trace time.
  - Per output channel: ScalarE does the per-triple seed subtract (Copy+bias),
    VectorE does 2 fused (v - w) max acc ops per triple (scalar_tensor_tensor)
    and the min over triples as tensor_tensor ops batched over groups of 32
    channels. (GPSIMD/PE/DMA-compute all measured slower for these op sizes.)
  - w2 folded into w1 (w1p = w1 + w2[triple]) -> max abs err ~2.4e-7.
  - Measured: ~151 us HW exec per core, rel err 6.8e-8; DVE and ACT both
    ~130us busy (balanced), plus ~15us input-DMA ramp and ~12us exit drain.
"""

import sys
import numpy as np

sys.path.insert(0, "/opt/trn_rl_repo")

B, C, H, W = 16, 64, 64, 64
O = 128
NCORES = 8
BL = B // NCORES          # batches per core
WP = W + 2                # padded width
FREE = C * WP             # per-partition free size of xs (centered copy)
GO = 32                   # output channels per min-stage group

_cache = {}


def _build_program(c_, kh, kw, w1p):
    """Build + compile the SPMD bass program. Gather offsets and weights are
    baked into the instruction stream as immediates."""
    from contextlib import ExitStack
    import concourse.tile as tile
    from concourse import bacc, mybir

    f32 = mybir.dt.float32
    Alu = mybir.AluOpType
    Act = mybir.ActivationFunctionType

    nc = bacc.Bacc("TRN2", target_bir_lowering=False, debug=False,
                   num_devices=NCORES)
    xs_d = nc.dram_tensor("xs", [128, FREE], f32, kind="ExternalInput")
    y_d = nc.dram_tensor("y", [128, O * W], f32, kind="ExternalOutput")

    with tile.TileContext(nc) as tc, ExitStack() as ctx:
        xs_pool = ctx.enter_context(tc.tile_pool(name="xs", bufs=1))
        t_pool = ctx.enter_context(tc.tile_pool(name="t", bufs=24))
        m_pool = ctx.enter_context(tc.tile_pool(name="m", bufs=24))
        ma_pool = ctx.enter_context(tc.tile_pool(name="ma", bufs=3))
        r_pool = ctx.enter_context(tc.tile_pool(name="r", bufs=3))
        o_pool = ctx.enter_context(tc.tile_pool(name="o", bufs=4))

        # Warm the ACT function table while the input DMA is in flight.
        warm_t = t_pool.tile([128, 8], f32, tag="warm")
        nc.gpsimd.memset(warm_t[:], 0.0)
        nc.scalar.activation(warm_t[:], warm_t[:], Act.Copy, bias=0.0,
                             scale=1.0)

        # Only the centered (kh=1) copy is loaded from HBM, as (c-block)
        # sub-tiles; the kh=0/2 h-shifted copies are built on-chip with
        # partition-shifted SBUF->SBUF DMAs. Edge padding makes the border
        # rows of the shifted copies equal to the source border rows, so the
        # clamp is a 1-partition self-copy.
        CB = 16                       # channels per sub-tile
        NCB = C // CB
        sub_sz = CB * WP
        xs_ts = {}
        for cb in range(NCB):
            xt = xs_pool.tile([128, sub_sz], f32, tag=f"xs1_{cb}")
            xs_ts[(1, cb)] = xt
            nc.sync.dma_start(xt[:], xs_d[:, cb * sub_sz:(cb + 1) * sub_sz])
        for cb in range(NCB):
            src = xs_ts[(1, cb)]
            up = xs_pool.tile([128, sub_sz], f32, tag=f"xs0_{cb}")
            dn = xs_pool.tile([128, sub_sz], f32, tag=f"xs2_{cb}")
            xs_ts[(0, cb)] = up
            xs_ts[(2, cb)] = dn
            for b0 in (0, 64):
                # kh=0: dest row h reads source row h-1; row 0 clamps.
                nc.sync.dma_start(up[b0 + 1:b0 + 64, :],
                                  src[b0:b0 + 63, :])
                nc.sync.dma_start(up[b0:b0 + 1, :], src[b0:b0 + 1, :])
                # kh=2: dest row h reads source row h+1; row 63 clamps.
                nc.sync.dma_start(dn[b0:b0 + 63, :],
                                  src[b0 + 1:b0 + 64, :])
                nc.sync.dma_start(dn[b0 + 63:b0 + 64, :],
                                  src[b0 + 63:b0 + 64, :])

        def vslice(o, j):
            d, c, k = kh[o, j], c_[o, j], kw[o, j]
            xt = xs_ts[(d, c // CB)]
            base = (c % CB) * WP + k
            return xt[:, base:base + W]

        # Process channels ordered by when their last-needed sub-tile is
        # ready (shifted copies land after their source c-block). Host
        # unpermutes columns.
        def rank(o):
            return max(c_[o, j] // CB + (0 if kh[o, j] == 1 else NCB)
                       for j in range(9))
        order = np.argsort([rank(o) for o in range(O)], kind="stable")

        for og in range(O // GO):
            ma_t = ma_pool.tile([128, GO * 3 * W], f32)
            for ol in range(GO):
                o = int(order[og * GO + ol])
                for i in range(3):
                    j0, j1, j2 = 3 * i, 3 * i + 1, 3 * i + 2
                    t_t = t_pool.tile([128, W], f32)
                    nc.scalar.activation(t_t[:], vslice(o, j0), Act.Copy,
                                         bias=-float(w1p[o, j0]),
                                         scale=1.0)
                    m_t = m_pool.tile([128, W], f32)
                    nc.vector.scalar_tensor_tensor(
                        m_t[:], vslice(o, j1), float(w1p[o, j1]), t_t[:],
                        op0=Alu.subtract, op1=Alu.max)
                    ma_sl = ma_t[:, (ol * 3 + i) * W:(ol * 3 + i + 1) * W]
                    nc.vector.scalar_tensor_tensor(
                        ma_sl, vslice(o, j2), float(w1p[o, j2]), m_t[:],
                        op0=Alu.subtract, op1=Alu.max)
            mav = ma_t[:].rearrange("p (o i w) -> p o i w", o=GO, i=3)
            r_t = r_pool.tile([128, GO * W], f32)
            rv = r_t[:].rearrange("p (o w) -> p o w", o=GO)
            nc.vector.tensor_tensor(rv, mav[:, :, 0, :], mav[:, :, 1, :],
                                    Alu.min)
            out_t = o_pool.tile([128, GO * W], f32)
            ov = out_t[:].rearrange("p (o w) -> p o w", o=GO)
            nc.vector.tensor_tensor(ov, rv, mav[:, :, 2, :], Alu.min)
            nc.sync.dma_start(y_d[:, og * GO * W:(og + 1) * GO * W], out_t[:])

    nc.compile()
    return nc, order


def _get_program(conn, w1p):
    key = (conn.tobytes(), w1p.tobytes())
    if key not in _cache:
        conn2 = conn.reshape(O, 9)
        c_ = (conn2 // 9).astype(np.int64)
        kh = ((conn2 % 9) // 3).astype(np.int64)
        kw = (conn2 % 3).astype(np.int64)
        _cache[key] = _build_program(c_, kh, kw, w1p)
    return _cache[key]


def kernel(x, w1, w2, conn, _trace=False, _trace_kwargs=None):
    x = np.ascontiguousarray(np.asarray(x, dtype=np.float32))
    w1 = np.asarray(w1, dtype=np.float32)
    w2 = np.asarray(w2, dtype=np.float32)
    conn = np.asarray(conn, dtype=np.int32)

    w1p = (w1 + np.repeat(w2, 3, axis=1)).astype(np.float32)
    nc, order = _get_program(conn, w1p)

    # Host prep: w-edge-padded centered copy, laid out [b*64+h, c, w_pad]
    # per core (the h-shifted copies are built on-chip).
    xw = np.pad(x, ((0, 0), (0, 0), (0, 0), (1, 1)), mode="edge")
    # -> [B, H, C, WP]
    sh = xw.transpose(0, 2, 1, 3)
    in_maps = []
    for k in range(NCORES):
        xs_core = np.ascontiguousarray(
            sh[BL * k:BL * (k + 1)].reshape(BL * H, FREE), dtype=np.float32)
        in_maps.append({"xs": xs_core})

    from concourse.bass_utils import run_bass_kernel_spmd
    res = run_bass_kernel_spmd(nc, in_maps, core_ids=list(range(NCORES)),
                               trace=_trace, **(_trace_kwargs or {}))

    out = np.empty((B, O, H, W), dtype=np.float32)
    for k in range(NCORES):
        yk = res.results[k]["y"]  # [128, O*W], o-columns in `order`
        tmp = yk.reshape(BL, H, O, W).transpose(0, 2, 1, 3)
        out[BL * k:BL * (k + 1), order] = tmp
    if _trace:
        kernel._last_results = res
    return out


# revision 22
# speedup vs baseline: 1.0403x; 1.0403x over previous
"""Trainium2 Bass kernel for nn_Minimax_Conv2D.

Semantics (reference): for each output channel o and pixel (b,h,w):
    v_j = x_padEdge[b, c_j, h+kh_j, w+kw_j]   (c_j,kh_j,kw_j) = decode(conn[o*9+j])
    out  = min_i max_{j in triple i} (v_j - w1[o,j]) - w2[o,i]

Strategy:
  - 8-way data parallel over batch (2 batches/core), identical SPMD program.
  - Per core SBUF layout: partitions p = b_local*64 + h ; free = (dh, c, w_pad)
    holding 3 h-shifted edge-padded copies of the input, so every gather
    offset (c, kh, kw) is a static free-dim slice baked at trace time.
  - Per output channel: ScalarE/GPSIMD do the per-triple seed subtract,
    VectorE does 2 fused (v - w) max acc ops per triple (scalar_tensor_tensor)
    and the min over triples as tensor_tensor ops batched over groups of 32
    channels.
  - w2 folded into w1 (w1p = w1 + w2[triple]) -> max abs err ~2.4e-7.
  - Measured: ~153 us HW exec per core, rel err 6.8e-8.
"""

import sys
import numpy as np

sys.path.insert(0, "/opt/trn_rl_repo")

B, C, H, W = 16, 64, 64, 64
O = 128
NCORES = 8
BL = B // NCORES          # batches per core
WP = W + 2                # padded width
FREE = 3 * C * WP         # per-partition free size of xs
GO = 32                   # output channels per min-stage group

_cache = {}


def _build_program(c_, kh, kw, w1p):
    """Build + compile the SPMD bass program. Gather offsets and weights are
    baked into the instruction stream as immediates."""
    from contextlib import ExitStack
    import concourse.tile as tile
    from concourse import bacc, mybir

    f32 = mybir.dt.float32
    Alu = mybir.AluOpType
    Act = mybir.ActivationFunctionType

    nc = bacc.Bacc("TRN2", target_bir_lowering=False, debug=False,
                   num_devices=NCORES)
    xs_d = nc.dram_tensor("xs", [128, FREE], f32, kind="ExternalInput")
    y_d = nc.dram_tensor("y", [128, O * W], f32, kind="ExternalOutput")

    with tile.TileContext(nc) as tc, ExitStack() as ctx:
        xs_pool = ctx.enter_context(tc.tile_pool(name="xs", bufs=1))
        t_pool = ctx.enter_context(tc.tile_pool(name="t", bufs=24))
        m_pool = ctx.enter_context(tc.tile_pool(name="m", bufs=24))
        ma_pool = ctx.enter_context(tc.tile_pool(name="ma", bufs=3))
        r_pool = ctx.enter_context(tc.tile_pool(name="r", bufs=3))
        o_pool = ctx.enter_context(tc.tile_pool(name="o", bufs=4))

        # Warm the ACT function table while the input DMA is in flight.
        warm_t = t_pool.tile([128, 8], f32, tag="warm")
        nc.gpsimd.memset(warm_t[:], 0.0)
        nc.scalar.activation(warm_t[:], warm_t[:], Act.Copy, bias=0.0,
                             scale=1.0)

        # xs split into (dh, c-block) sub-tiles so compute can start before
        # the whole 6.5MB input lands.
        CB = 16                       # channels per sub-tile
        NSUB = 3 * (C // CB)
        sub_sz = CB * WP
        xs_ts = []
        for s in range(NSUB):
            xt = xs_pool.tile([128, sub_sz], f32, tag=f"xs{s}")
            eng = nc.sync if s % 2 == 0 else nc.scalar
            eng.dma_start(xt[:], xs_d[:, s * sub_sz:(s + 1) * sub_sz])
            xs_ts.append(xt)

        def vslice(o, j):
            d, c, k = kh[o, j], c_[o, j], kw[o, j]
            xt = xs_ts[d * (C // CB) + c // CB]
            base = (c % CB) * WP + k
            return xt[:, base:base + W]

        # Process channels ordered by the last xs sub-tile they touch, so
        # early channels only wait on early DMAs. Host unpermutes columns.
        order = np.argsort(
            [max(kh[o, j] * (C // CB) + c_[o, j] // CB for j in range(9))
             for o in range(O)], kind="stable")

        for og in range(O // GO):
            ma_t = ma_pool.tile([128, GO * 3 * W], f32)
            for ol in range(GO):
                o = int(order[og * GO + ol])
                for i in range(3):
                    j0, j1, j2 = 3 * i, 3 * i + 1, 3 * i + 2
                    t_t = t_pool.tile([128, W], f32)
                    nc.scalar.activation(t_t[:], vslice(o, j0), Act.Copy,
                                         bias=-float(w1p[o, j0]),
                                         scale=1.0)
                    m_t = m_pool.tile([128, W], f32)
                    nc.vector.scalar_tensor_tensor(
                        m_t[:], vslice(o, j1), float(w1p[o, j1]), t_t[:],
                        op0=Alu.subtract, op1=Alu.max)
                    ma_sl = ma_t[:, (ol * 3 + i) * W:(ol * 3 + i + 1) * W]
                    nc.vector.scalar_tensor_tensor(
                        ma_sl, vslice(o, j2), float(w1p[o, j2]), m_t[:],
                        op0=Alu.subtract, op1=Alu.max)
            mav = ma_t[:].rearrange("p (o i w) -> p o i w", o=GO, i=3)
            r_t = r_pool.tile([128, GO * W], f32)
            rv = r_t[:].rearrange("p (o w) -> p o w", o=GO)
            nc.vector.tensor_tensor(rv, mav[:, :, 0, :], mav[:, :, 1, :],
                                    Alu.min)
            out_t = o_pool.tile([128, GO * W], f32)
            ov = out_t[:].rearrange("p (o w) -> p o w", o=GO)
            nc.vector.tensor_tensor(ov, rv, mav[:, :, 2, :], Alu.min)
            nc.sync.dma_start(y_d[:, og * GO * W:(og + 1) * GO * W], out_t[:])

    nc.compile()
    return nc, order


def _get_program(conn, w1p):
    key = (conn.tobytes(), w1p.tobytes())
    if key not in _cache:
        conn2 = conn.reshape(O, 9)
        c_ = (conn2 // 9).astype(np.int64)
        kh = ((conn2 % 9) // 3).astype(np.int64)
        kw = (conn2 % 3).astype(np.int64)
        _cache[key] = _build_program(c_, kh, kw, w1p)
    return _cache[key]


def kernel(x, w1, w2, conn, _trace=False, _trace_kwargs=None):
    x = np.ascontiguousarray(np.asarray(x, dtype=np.float32))
    w1 = np.asarray(w1, dtype=np.float32)
    w2 = np.asarray(w2, dtype=np.float32)
    conn = np.asarray(conn, dtype=np.int32)

    w1p = (w1 + np.repeat(w2, 3, axis=1)).astype(np.float32)
    nc, order = _get_program(conn, w1p)

    # Host prep: 3 h-shifted edge-padded copies, laid out
    # [b*64+h, dh, c, w_pad] per core.
    xp = np.pad(x, ((0, 0), (0, 0), (1, 1), (1, 1)), mode="edge")
    # [B, C, 3, 64, 66]
    sh = np.stack([xp[:, :, d:d + H, :] for d in range(3)], axis=2)
    # -> [B, H, 3, C, WP]
    sh = sh.transpose(0, 3, 2, 1, 4)
    in_maps = []
    for k in range(NCORES):
        xs_core = np.ascontiguousarray(
            sh[BL * k:BL * (k + 1)].reshape(BL * H, FREE), dtype=np.float32)
        in_maps.append({"xs": xs_core})

    from concourse.bass_utils import run_bass_kernel_spmd
    res = run_bass_kernel_spmd(nc, in_maps, core_ids=list(range(NCORES)),
                               trace=_trace, **(_trace_kwargs or {}))

    out = np.empty((B, O, H, W), dtype=np.float32)
    for k in range(NCORES):
        yk = res.results[k]["y"]  # [128, O*W], o-columns in `order`
        tmp = yk.reshape(BL, H, O, W).transpose(0, 2, 1, 3)
        out[BL * k:BL * (k + 1), order] = tmp
    if _trace:
        kernel._last_results = res
    return out
